# revision 6
# baseline (speedup 1.0000x reference)
"""GCN encoder (2-layer, with feature concat) as a Bass/Trainium2 SPMD kernel.

Math (equivalent to the reference, by linearity of segment_sum):
    T1 = x @ A1                where A1 = W_gc1 @ W_th[:200]
    g1 = segsum(w * T1[src])   (dst-sorted scatter-add)
    h1 = relu(g1 + x @ B1)     where B1 = W_th[200:]
    T2 = h1 @ A2               where A2 = W_gc2 @ W_th1[:200]
    g2 = segsum(w * T2[src])
    h2 = relu(g2 + x @ B2)     where B2 = W_th1[200:]
    returns (h1, h2)

Sharding: nodes (dst side) split across 8 cores, 12500 each (padded to
98 blocks x 128).  Each core computes T1/T2 rows for its own nodes, the
table is replicated via AllGather, and each core processes the edges
pointing at its own nodes: per 128-node block, per-edge rows are fetched
from the table with [P,1] indirect DMAs (128 rows/call) and accumulated
into PSUM with selection-matrix matmuls (sel[e,d] = w_e * (dst_e == d)),
built on the fly by the vector engine from an iota tile.
"""

import sys

sys.path.insert(0, "/opt/trn_rl_repo")

import numpy as np

N_NODES = 100000
N_EDGES = 3200000
N_FEAT = 512
HID = 200
NC = 8
NPC_RAW = N_NODES // NC          # 12500
BLK = 128
NBLK = (NPC_RAW + BLK - 1) // BLK  # 98
NPC = NBLK * BLK                 # 12544
ROW = HID                        # table row = 200 fp32 = 800 B
FCH = N_FEAT // 128              # 4 feature chunks

_compiled_cache: dict = {}


def _build(nc_mod, T_b, nblk=None, npc=None, nrow_full=None):
    """Build the SPMD Bass program. T_b = gather tiles per block (uniform)."""
    import concourse.bacc as bacc
    import concourse.bass as bass
    import concourse.tile as tile
    import concourse.mybir as mybir
    from contextlib import ExitStack

    dt = mybir.dt
    P = 128
    if nblk is None:
        nblk = NBLK
    if npc is None:
        npc = NPC
    if nrow_full is None:
        nrow_full = NC * npc
    TT = nblk * T_b

    nc = bacc.Bacc("TRN2", target_bir_lowering=False, debug=False, num_devices=nc_mod)

    # ---- inputs ----
    xT = nc.declare_dram_parameter("xT", [FCH, P, npc], dt.float32, isOutput=False)
    idx = nc.declare_dram_parameter("idx", [P, TT], dt.int32, isOutput=False)
    dlc = nc.declare_dram_parameter("dlc", [P, TT], dt.float32, isOutput=False)
    wts = nc.declare_dram_parameter("wts", [P, TT], dt.float32, isOutput=False)
    A1 = nc.declare_dram_parameter("A1", [FCH, P, HID], dt.float32, isOutput=False)
    B1 = nc.declare_dram_parameter("B1", [FCH, P, HID], dt.float32, isOutput=False)
    B2 = nc.declare_dram_parameter("B2", [FCH, P, HID], dt.float32, isOutput=False)
    A2 = nc.declare_dram_parameter("A2", [2, P, HID], dt.float32, isOutput=False)
    iota = nc.declare_dram_parameter("iota", [P, P], dt.float32, isOutput=False)
    ident = nc.declare_dram_parameter("ident", [P, P], dt.float32, isOutput=False)
    out1 = nc.declare_dram_parameter("out1", [npc, HID], dt.float32, isOutput=True)
    out2 = nc.declare_dram_parameter("out2", [npc, HID], dt.float32, isOutput=True)

    # ---- internal DRAM ----
    T1own = nc.dram_tensor("T1own", [npc, ROW], dt.float32)
    T1full = nc.dram_tensor("T1full", [nrow_full, ROW], dt.float32, addr_space="Shared")
    T2own = nc.dram_tensor("T2own", [npc, ROW], dt.float32)
    T2full = nc.dram_tensor("T2full", [nrow_full, ROW], dt.float32, addr_space="Shared")
    xb1d = nc.dram_tensor("xb1d", [npc, ROW], dt.float32)
    xb2d = nc.dram_tensor("xb2d", [npc, ROW], dt.float32)

    eq = mybir.AluOpType.is_equal
    mul = mybir.AluOpType.mult
    relu = mybir.ActivationFunctionType.Relu

    with tile.TileContext(nc) as tc, ExitStack() as ctx:
        const_pool = ctx.enter_context(tc.tile_pool(name="const", bufs=1))
        iota_t = const_pool.tile([P, P], dt.float32)
        nc.sync.dma_start(iota_t[:], iota[:])
        ident_t = const_pool.tile([P, P], dt.float32)
        nc.sync.dma_start(ident_t[:], ident[:])
        idx_t = const_pool.tile([P, TT], dt.int32)
        nc.sync.dma_start(idx_t[:], idx[:])
        dlc_t = const_pool.tile([P, TT], dt.float32)
        nc.sync.dma_start(dlc_t[:], dlc[:])
        wts_t = const_pool.tile([P, TT], dt.float32)
        nc.sync.dma_start(wts_t[:], wts[:])
        A1_t = const_pool.tile([P, FCH * HID], dt.float32)
        B1_t = const_pool.tile([P, FCH * HID], dt.float32)
        B2_t = const_pool.tile([P, FCH * HID], dt.float32)
        A2_t = const_pool.tile([P, 2 * HID], dt.float32)
        for c in range(FCH):
            nc.sync.dma_start(A1_t[:, c * HID:(c + 1) * HID], A1[c])
            nc.sync.dma_start(B1_t[:, c * HID:(c + 1) * HID], B1[c])
            nc.sync.dma_start(B2_t[:, c * HID:(c + 1) * HID], B2[c])
        for c in range(2):
            nc.sync.dma_start(A2_t[:, c * HID:(c + 1) * HID], A2[c])

        def wslice(t, c):
            return t[:, c * HID:(c + 1) * HID]

        # ---- phase 1: T1 = x@A1, xb1 = x@B1, xb2 = x@B2 (own rows) ----
        with tc.tile_pool(name="p1x", bufs=2) as p1x, \
             tc.tile_pool(name="p1o", bufs=3) as p1o, \
             tc.tile_pool(name="p1ps", bufs=2, space="PSUM") as p1ps:
            SB = 4  # blocks per xT load
            for g in range((nblk + SB - 1) // SB):
                b_lo = g * SB
                b_hi = min(b_lo + SB, nblk)
                span = (b_hi - b_lo) * BLK
                xts = []
                for c in range(FCH):
                    xt = p1x.tile([P, SB * BLK], dt.float32, tag=f"xt{c}")
                    nc.sync.dma_start(xt[:, :span], xT[c, :, b_lo * BLK:b_hi * BLK])
                    xts.append(xt)
                for b in range(b_lo, b_hi):
                    off = (b - b_lo) * BLK
                    ps_t1 = p1ps.tile([P, HID], dt.float32, tag="t1")
                    ps_x1 = p1ps.tile([P, HID], dt.float32, tag="x1")
                    ps_x2 = p1ps.tile([P, HID], dt.float32, tag="x2")
                    for c in range(FCH):
                        lhs = xts[c][:, off:off + BLK]
                        nc.tensor.matmul(ps_t1[:], lhs, wslice(A1_t, c), start=(c == 0), stop=(c == FCH - 1))
                        nc.tensor.matmul(ps_x1[:], lhs, wslice(B1_t, c), start=(c == 0), stop=(c == FCH - 1))
                        nc.tensor.matmul(ps_x2[:], lhs, wslice(B2_t, c), start=(c == 0), stop=(c == FCH - 1))
                    r = slice(b * BLK, (b + 1) * BLK)
                    t1s = p1o.tile([P, ROW], dt.float32, tag="t1s")
                    nc.vector.tensor_copy(t1s[:], ps_t1[:])
                    nc.sync.dma_start(T1own[r, :], t1s[:])
                    x1s = p1o.tile([P, ROW], dt.float32, tag="x1s")
                    nc.scalar.copy(x1s[:], ps_x1[:])
                    nc.sync.dma_start(xb1d[r, :], x1s[:])
                    x2s = p1o.tile([P, ROW], dt.float32, tag="x2s")
                    nc.vector.tensor_copy(x2s[:], ps_x2[:])
                    nc.sync.dma_start(xb2d[r, :], x2s[:])

        nc.gpsimd.collective_compute(
            "AllGather", mybir.AluOpType.bypass,
            replica_groups=[list(range(nc_mod))],
            ins=[T1own[:]], outs=[T1full[:]],
        )

        # ---- edge layers ----
        def edge_layer(table, xbd, out_h, produce_t2):
            with tc.tile_pool(name="msgs", bufs=12) as msgs_pool, \
                 tc.tile_pool(name="sel", bufs=6) as sel_pool, \
                 tc.tile_pool(name="hb", bufs=4) as h_pool, \
                 tc.tile_pool(name="xb", bufs=3) as xb_pool, \
                 tc.tile_pool(name="gps", bufs=2, space="PSUM") as gps, \
                 tc.tile_pool(name="tps", bufs=2, space="PSUM") as tps:
                for b in range(nblk):
                    xbt = xb_pool.tile([P, ROW], dt.float32, tag="xbt")
                    nc.sync.dma_start(xbt[:], xbd[b * BLK:(b + 1) * BLK, :])
                    ps = gps.tile([P, HID], dt.float32, tag="g")
                    nc.tensor.matmul(ps[:], ident_t[:], xbt[:], start=True, stop=False)
                    for t in range(T_b):
                        gt = b * T_b + t
                        m = msgs_pool.tile([P, ROW], dt.float32, tag="m")
                        nc.gpsimd.indirect_dma_start(
                            out=m[:], out_offset=None, in_=table[:],
                            in_offset=bass.IndirectOffsetOnAxis(ap=idx_t[:, gt:gt + 1], axis=0),
                        )
                        sel = sel_pool.tile([P, P], dt.float32, tag="s")
                        nc.vector.tensor_scalar(
                            out=sel[:], in0=iota_t[:],
                            scalar1=dlc_t[:, gt:gt + 1], scalar2=wts_t[:, gt:gt + 1],
                            op0=eq, op1=mul,
                        )
                        nc.tensor.matmul(ps[:], sel[:], m[:], start=False, stop=(t == T_b - 1))
                    hf = h_pool.tile([P, HID], dt.float32, tag="hf")
                    nc.scalar.activation(hf[:], ps[:], relu)
                    nc.sync.dma_start(out_h[b * BLK:(b + 1) * BLK, :], hf[:])
                    if produce_t2:
                        tr0 = tps.tile([P, P], dt.float32, tag="tr")
                        nc.tensor.transpose(tr0[:], hf[:, 0:P], ident_t[:])
                        tr1 = tps.tile([P, P], dt.float32, tag="tr")
                        nc.tensor.transpose(tr1[:HID - P, :], hf[:, P:HID], ident_t[:])
                        hT0 = h_pool.tile([P, P], dt.float32, tag="hT0")
                        nc.vector.tensor_copy(hT0[:], tr0[:])
                        hT1 = h_pool.tile([P, P], dt.float32, tag="hT1")
                        nc.vector.tensor_copy(hT1[:HID - P, :], tr1[:HID - P, :])
                        t2ps = tps.tile([P, HID], dt.float32, tag="t2")
                        nc.tensor.matmul(t2ps[:], hT0[:], wslice(A2_t, 0), start=True, stop=False)
                        nc.tensor.matmul(t2ps[:], hT1[:HID - P, :], wslice(A2_t, 1)[:HID - P, :], start=False, stop=True)
                        t2s = h_pool.tile([P, ROW], dt.float32, tag="t2s")
                        nc.vector.tensor_copy(t2s[:], t2ps[:])
                        nc.sync.dma_start(T2own[b * BLK:(b + 1) * BLK, :], t2s[:])

        edge_layer(T1full, xb1d, out1, produce_t2=True)

        nc.gpsimd.collective_compute(
            "AllGather", mybir.AluOpType.bypass,
            replica_groups=[list(range(nc_mod))],
            ins=[T2own[:]], outs=[T2full[:]],
        )

        edge_layer(T2full, xb2d, out2, produce_t2=False)

    nc.compile()
    return nc


def _prep_host(features, src, dst, edge_weight, W_gc1, W_gc2, W_th, W_th1):
    """Host-side preprocessing: fused weights, per-core transposed feature
    slices, and per-core edge streams (gather indices + selection operands)."""
    f32 = np.float32
    A1 = (W_gc1 @ W_th[:HID]).astype(f32)
    B1 = np.ascontiguousarray(W_th[HID:]).astype(f32)
    A2 = (W_gc2 @ W_th1[:HID]).astype(f32)
    B2 = np.ascontiguousarray(W_th1[HID:]).astype(f32)
    A1c = np.ascontiguousarray(A1.reshape(FCH, 128, HID))
    B1c = np.ascontiguousarray(B1.reshape(FCH, 128, HID))
    B2c = np.ascontiguousarray(B2.reshape(FCH, 128, HID))
    A2p = np.zeros((2, 128, HID), f32)
    A2p[0] = A2[:128]
    A2p[1, :HID - 128] = A2[128:]

    iota = np.tile(np.arange(128, dtype=f32), (128, 1))
    ident = np.eye(128, dtype=f32)

    table_row = (src // NPC_RAW).astype(np.int64) * NPC + (src % NPC_RAW)

    per_core = []
    counts_all = []
    for c in range(NC):
        lo = np.searchsorted(dst, NPC_RAW * c)
        hi = np.searchsorted(dst, NPC_RAW * (c + 1))
        d_loc = (dst[lo:hi] - NPC_RAW * c).astype(np.int64)
        blk = d_loc >> 7
        blk_starts = np.searchsorted(d_loc, np.arange(NBLK) * BLK)
        cnts = np.diff(np.append(blk_starts, hi - lo))
        counts_all.append(cnts)
        per_core.append((lo, hi, d_loc, blk, blk_starts))
    T_b = int(max((np.ceil(c / BLK).astype(int).max() for c in counts_all)))
    TT = NBLK * T_b

    in_maps = []
    for c in range(NC):
        lo, hi, d_loc, blk, blk_starts = per_core[c]
        n_e = hi - lo
        idx_flat = np.zeros(TT * 128, np.int32)
        dl_flat = np.zeros(TT * 128, f32)
        w_flat = np.zeros(TT * 128, f32)
        pos = np.arange(n_e) - blk_starts[blk]
        slot = blk * (T_b * 128) + pos
        idx_flat[slot] = table_row[lo:hi].astype(np.int32)
        dl_flat[slot] = (d_loc & 127).astype(f32)
        w_flat[slot] = edge_weight[lo:hi].astype(f32)
        idx_dev = np.ascontiguousarray(idx_flat.reshape(TT, 128).T)
        dl_dev = np.ascontiguousarray(dl_flat.reshape(TT, 128).T)
        w_dev = np.ascontiguousarray(w_flat.reshape(TT, 128).T)

        xf = np.zeros((NPC, N_FEAT), f32)
        xf[:NPC_RAW] = features[NPC_RAW * c:NPC_RAW * (c + 1)]
        xTc = np.ascontiguousarray(xf.T.reshape(FCH, 128, NPC))

        in_maps.append({
            "xT": xTc, "idx": idx_dev, "dlc": dl_dev, "wts": w_dev,
            "A1": A1c, "B1": B1c, "B2": B2c, "A2": A2p,
            "iota": iota, "ident": ident,
        })
    return in_maps, T_b


last_exec_time_ns = None
last_results = None


def kernel(features, src, dst, edge_weight, W_gc1, W_gc2, W_th, W_th1,
           trace=False, tmpdir=None):
    global last_exec_time_ns, last_results
    from concourse.bass_utils import run_bass_kernel_spmd

    features = np.asarray(features, np.float32)
    src = np.asarray(src, np.int32)
    dst = np.asarray(dst, np.int32)
    edge_weight = np.asarray(edge_weight, np.float32)
    W_gc1 = np.asarray(W_gc1, np.float32)
    W_gc2 = np.asarray(W_gc2, np.float32)
    W_th = np.asarray(W_th, np.float32)
    W_th1 = np.asarray(W_th1, np.float32)

    in_maps, T_b = _prep_host(features, src, dst, edge_weight,
                              W_gc1, W_gc2, W_th, W_th1)

    key = T_b
    if key not in _compiled_cache:
        _compiled_cache[key] = _build(NC, T_b)
    nc = _compiled_cache[key]

    res = run_bass_kernel_spmd(nc, in_maps, list(range(NC)),
                               trace=trace, tmpdir=tmpdir)
    last_exec_time_ns = res.exec_time_ns
    last_results = res

    h1 = np.empty((N_NODES, HID), np.float32)
    h2 = np.empty((N_NODES, HID), np.float32)
    for c in range(NC):
        h1[NPC_RAW * c:NPC_RAW * (c + 1)] = res.results[c]["out1"][:NPC_RAW]
        h2[NPC_RAW * c:NPC_RAW * (c + 1)] = res.results[c]["out2"][:NPC_RAW]
    return (h1, h2)


# revision 9
# speedup vs baseline: 1.1188x; 1.1188x over previous
"""GCN encoder (2-layer, with feature concat) as a Bass/Trainium2 SPMD kernel.

Math (equivalent to the reference, by linearity of segment_sum):
    T1 = x @ A1                where A1 = W_gc1 @ W_th[:200]
    g1 = segsum(w * T1[src])   (dst-sorted scatter-add)
    h1 = relu(g1 + x @ B1)     where B1 = W_th[200:]
    T2 = h1 @ A2               where A2 = W_gc2 @ W_th1[:200]
    g2 = segsum(w * T2[src])
    h2 = relu(g2 + x @ B2)     where B2 = W_th1[200:]
    returns (h1, h2)

Sharding: nodes (dst side) split across 8 cores, 12500 each (padded to
98 blocks x 128).  Each core computes T1/T2 rows for its own nodes, the
table is replicated via AllGather, and each core processes the edges
pointing at its own nodes: per 128-node block, per-edge rows are fetched
from the table with [P,1] indirect DMAs (128 rows/call, the only working
gather primitive on this stack) and accumulated into PSUM with
selection-matrix matmuls (sel[e,d] = w_e * (dst_e == d)), built on the
fly by the vector engine from an iota tile.  PE operands are fp16 (PSUM
accumulation is fp32); outputs are exact fp32 relu copies of PSUM.
"""

import sys

sys.path.insert(0, "/opt/trn_rl_repo")

import numpy as np

N_NODES = 100000
N_EDGES = 3200000
N_FEAT = 512
HID = 200
NC = 8
NPC_RAW = N_NODES // NC          # 12500
BLK = 128
NBLK = (NPC_RAW + BLK - 1) // BLK  # 98
NPC = NBLK * BLK                 # 12544
ROW = HID                        # table row = 200 elems
FCH = N_FEAT // 128              # 4 feature chunks

_compiled_cache: dict = {}


def _build(nc_mod, tbs, nblk=None, npc=None, nrow_full=None):
    """Build the SPMD Bass program. tbs[b] = gather tiles for block b
    (same for all cores; data-dependent padding is in the index streams)."""
    import concourse.bacc as bacc
    import concourse.bass as bass
    import concourse.tile as tile
    import concourse.mybir as mybir
    from contextlib import ExitStack

    dt = mybir.dt
    P = 128
    if nblk is None:
        nblk = NBLK
    if npc is None:
        npc = NPC
    if nrow_full is None:
        nrow_full = NC * npc
    assert len(tbs) == nblk
    TT = int(np.sum(tbs))
    toff = np.concatenate([[0], np.cumsum(tbs)]).astype(int)
    f16 = dt.float16
    f32 = dt.float32

    nc = bacc.Bacc("TRN2", target_bir_lowering=False, debug=False, num_devices=nc_mod)

    # ---- inputs ----
    xT = nc.declare_dram_parameter("xT", [FCH, P, npc], f16, isOutput=False)
    idx = nc.declare_dram_parameter("idx", [P, TT], dt.int32, isOutput=False)
    dlc = nc.declare_dram_parameter("dlc", [P, TT], f32, isOutput=False)
    wts = nc.declare_dram_parameter("wts", [P, TT], f32, isOutput=False)
    A1 = nc.declare_dram_parameter("A1", [FCH, P, HID], f16, isOutput=False)
    B1 = nc.declare_dram_parameter("B1", [FCH, P, HID], f16, isOutput=False)
    B2 = nc.declare_dram_parameter("B2", [FCH, P, HID], f16, isOutput=False)
    A2 = nc.declare_dram_parameter("A2", [2, P, HID], f16, isOutput=False)
    iota = nc.declare_dram_parameter("iota", [P, P], f16, isOutput=False)
    ident = nc.declare_dram_parameter("ident", [P, P], f16, isOutput=False)
    out1 = nc.declare_dram_parameter("out1", [npc, HID], f32, isOutput=True)
    out2 = nc.declare_dram_parameter("out2", [npc, HID], f32, isOutput=True)

    # ---- internal DRAM ----
    T1own = nc.dram_tensor("T1own", [npc, ROW], f16)
    T1full = nc.dram_tensor("T1full", [nrow_full, ROW], f16, addr_space="Shared")
    T2own = nc.dram_tensor("T2own", [npc, ROW], f16)
    T2full = nc.dram_tensor("T2full", [nrow_full, ROW], f16, addr_space="Shared")
    xb1d = nc.dram_tensor("xb1d", [npc, ROW], f16)
    xb2d = nc.dram_tensor("xb2d", [npc, ROW], f16)

    eq = mybir.AluOpType.is_equal
    mul = mybir.AluOpType.mult
    relu = mybir.ActivationFunctionType.Relu

    with tile.TileContext(nc) as tc, ExitStack() as ctx:
        const_pool = ctx.enter_context(tc.tile_pool(name="const", bufs=1))
        iota_t = const_pool.tile([P, P], f16)
        nc.sync.dma_start(iota_t[:], iota[:])
        ident_t = const_pool.tile([P, P], f16)
        nc.sync.dma_start(ident_t[:], ident[:])
        idx_t = const_pool.tile([P, TT], dt.int32)
        nc.sync.dma_start(idx_t[:], idx[:])
        dlc_t = const_pool.tile([P, TT], f32)
        nc.sync.dma_start(dlc_t[:], dlc[:])
        wts_t = const_pool.tile([P, TT], f32)
        nc.sync.dma_start(wts_t[:], wts[:])
        A1_t = const_pool.tile([P, FCH * HID], f16)
        B1_t = const_pool.tile([P, FCH * HID], f16)
        B2_t = const_pool.tile([P, FCH * HID], f16)
        A2_t = const_pool.tile([P, 2 * HID], f16)
        for c in range(FCH):
            nc.sync.dma_start(A1_t[:, c * HID:(c + 1) * HID], A1[c])
            nc.sync.dma_start(B1_t[:, c * HID:(c + 1) * HID], B1[c])
            nc.sync.dma_start(B2_t[:, c * HID:(c + 1) * HID], B2[c])
        for c in range(2):
            nc.sync.dma_start(A2_t[:, c * HID:(c + 1) * HID], A2[c])

        def wslice(t, c):
            return t[:, c * HID:(c + 1) * HID]

        # ---- phase 1: T1 = x@A1, xb1 = x@B1, xb2 = x@B2 (own rows) ----
        with tc.tile_pool(name="p1x", bufs=2) as p1x, \
             tc.tile_pool(name="p1o", bufs=3) as p1o, \
             tc.tile_pool(name="p1ps", bufs=2, space="PSUM") as p1ps:
            SB = 4  # blocks per xT load
            for g in range((nblk + SB - 1) // SB):
                b_lo = g * SB
                b_hi = min(b_lo + SB, nblk)
                span = (b_hi - b_lo) * BLK
                xts = []
                for c in range(FCH):
                    xt = p1x.tile([P, SB * BLK], f16, tag=f"xt{c}")
                    nc.sync.dma_start(xt[:, :span], xT[c, :, b_lo * BLK:b_hi * BLK])
                    xts.append(xt)
                for b in range(b_lo, b_hi):
                    off = (b - b_lo) * BLK
                    ps_t1 = p1ps.tile([P, HID], f32, tag="t1")
                    ps_x1 = p1ps.tile([P, HID], f32, tag="x1")
                    ps_x2 = p1ps.tile([P, HID], f32, tag="x2")
                    for c in range(FCH):
                        lhs = xts[c][:, off:off + BLK]
                        nc.tensor.matmul(ps_t1[:], lhs, wslice(A1_t, c), start=(c == 0), stop=(c == FCH - 1))
                        nc.tensor.matmul(ps_x1[:], lhs, wslice(B1_t, c), start=(c == 0), stop=(c == FCH - 1))
                        nc.tensor.matmul(ps_x2[:], lhs, wslice(B2_t, c), start=(c == 0), stop=(c == FCH - 1))
                    r = slice(b * BLK, (b + 1) * BLK)
                    t1s = p1o.tile([P, ROW], f16, tag="t1s")
                    nc.vector.tensor_copy(t1s[:], ps_t1[:])
                    nc.sync.dma_start(T1own[r, :], t1s[:])
                    x1s = p1o.tile([P, ROW], f16, tag="x1s")
                    nc.scalar.copy(x1s[:], ps_x1[:])
                    nc.sync.dma_start(xb1d[r, :], x1s[:])
                    x2s = p1o.tile([P, ROW], f16, tag="x2s")
                    nc.vector.tensor_copy(x2s[:], ps_x2[:])
                    nc.sync.dma_start(xb2d[r, :], x2s[:])

        nc.gpsimd.collective_compute(
            "AllGather", mybir.AluOpType.bypass,
            replica_groups=[list(range(nc_mod))],
            ins=[T1own[:]], outs=[T1full[:]],
        )

        # ---- edge layers ----
        def edge_layer(table, xbd, out_h, produce_t2):
            with tc.tile_pool(name="msgs", bufs=16) as msgs_pool, \
                 tc.tile_pool(name="sel", bufs=8) as sel_pool, \
                 tc.tile_pool(name="hb", bufs=4) as h_pool, \
                 tc.tile_pool(name="xb", bufs=3) as xb_pool, \
                 tc.tile_pool(name="gps", bufs=2, space="PSUM") as gps, \
                 tc.tile_pool(name="tps", bufs=2, space="PSUM") as tps:
                for b in range(nblk):
                    T_b = int(tbs[b])
                    xbt = xb_pool.tile([P, ROW], f16, tag="xbt")
                    nc.sync.dma_start(xbt[:], xbd[b * BLK:(b + 1) * BLK, :])
                    ps = gps.tile([P, HID], f32, tag="g")
                    nc.tensor.matmul(ps[:], ident_t[:], xbt[:], start=True, stop=False)
                    for t in range(T_b):
                        gt = int(toff[b]) + t
                        m = msgs_pool.tile([P, ROW], f16, tag="m")
                        nc.gpsimd.indirect_dma_start(
                            out=m[:], out_offset=None, in_=table[:],
                            in_offset=bass.IndirectOffsetOnAxis(ap=idx_t[:, gt:gt + 1], axis=0),
                        )
                        sel = sel_pool.tile([P, P], f16, tag="s")
                        nc.vector.tensor_scalar(
                            out=sel[:], in0=iota_t[:],
                            scalar1=dlc_t[:, gt:gt + 1], scalar2=wts_t[:, gt:gt + 1],
                            op0=eq, op1=mul,
                        )
                        nc.tensor.matmul(ps[:], sel[:], m[:], start=False, stop=(t == T_b - 1))
                    hf = h_pool.tile([P, HID], f32, tag="hf")
                    nc.scalar.activation(hf[:], ps[:], relu)
                    nc.sync.dma_start(out_h[b * BLK:(b + 1) * BLK, :], hf[:])
                    if produce_t2:
                        hf16 = h_pool.tile([P, HID], f16, tag="hf16")
                        nc.vector.tensor_copy(hf16[:], hf[:])
                        tr0 = tps.tile([P, P], f16, tag="tr")
                        nc.tensor.transpose(tr0[:], hf16[:, 0:P], ident_t[:])
                        tr1 = tps.tile([P, P], f16, tag="tr")
                        nc.tensor.transpose(tr1[:HID - P, :], hf16[:, P:HID], ident_t[:])
                        hT0 = h_pool.tile([P, P], f16, tag="hT0")
                        nc.vector.tensor_copy(hT0[:], tr0[:])
                        hT1 = h_pool.tile([P, P], f16, tag="hT1")
                        nc.vector.tensor_copy(hT1[:HID - P, :], tr1[:HID - P, :])
                        t2ps = tps.tile([P, HID], f32, tag="t2")
                        nc.tensor.matmul(t2ps[:], hT0[:], wslice(A2_t, 0), start=True, stop=False)
                        nc.tensor.matmul(t2ps[:], hT1[:HID - P, :], wslice(A2_t, 1)[:HID - P, :], start=False, stop=True)
                        t2s = h_pool.tile([P, ROW], f16, tag="t2s")
                        nc.vector.tensor_copy(t2s[:], t2ps[:])
                        nc.sync.dma_start(T2own[b * BLK:(b + 1) * BLK, :], t2s[:])

        edge_layer(T1full, xb1d, out1, produce_t2=True)

        nc.gpsimd.collective_compute(
            "AllGather", mybir.AluOpType.bypass,
            replica_groups=[list(range(nc_mod))],
            ins=[T2own[:]], outs=[T2full[:]],
        )

        edge_layer(T2full, xb2d, out2, produce_t2=False)

    nc.compile()
    return nc


def _prep_host(features, src, dst, edge_weight, W_gc1, W_gc2, W_th, W_th1):
    """Host-side preprocessing: fused weights, per-core transposed feature
    slices, and per-core edge streams (gather indices + selection operands)."""
    f32 = np.float32
    f16 = np.float16
    A1 = (W_gc1 @ W_th[:HID]).astype(f32)
    B1 = np.ascontiguousarray(W_th[HID:]).astype(f32)
    A2 = (W_gc2 @ W_th1[:HID]).astype(f32)
    B2 = np.ascontiguousarray(W_th1[HID:]).astype(f32)
    A1c = np.ascontiguousarray(A1.reshape(FCH, 128, HID)).astype(f16)
    B1c = np.ascontiguousarray(B1.reshape(FCH, 128, HID)).astype(f16)
    B2c = np.ascontiguousarray(B2.reshape(FCH, 128, HID)).astype(f16)
    A2p = np.zeros((2, 128, HID), f16)
    A2p[0] = A2[:128]
    A2p[1, :HID - 128] = A2[128:]

    iota = np.tile(np.arange(128, dtype=f16), (128, 1))
    ident = np.eye(128, dtype=f16)

    table_row = (src // NPC_RAW).astype(np.int64) * NPC + (src % NPC_RAW)

    per_core = []
    counts_all = []
    for c in range(NC):
        lo = np.searchsorted(dst, NPC_RAW * c)
        hi = np.searchsorted(dst, NPC_RAW * (c + 1))
        d_loc = (dst[lo:hi] - NPC_RAW * c).astype(np.int64)
        blk = d_loc >> 7
        blk_starts = np.searchsorted(d_loc, np.arange(NBLK) * BLK)
        cnts = np.diff(np.append(blk_starts, hi - lo))
        counts_all.append(cnts)
        per_core.append((lo, hi, d_loc, blk, blk_starts))
    # per-block tile count: max over cores (program is shared by all cores)
    cnts_mat = np.stack(counts_all)                      # [NC, NBLK]
    tbs = np.ceil(cnts_mat / BLK).astype(int).max(axis=0)
    tbs = np.maximum(tbs, 1)
    toff = np.concatenate([[0], np.cumsum(tbs)]).astype(np.int64)
    TT = int(toff[-1])

    in_maps = []
    for c in range(NC):
        lo, hi, d_loc, blk, blk_starts = per_core[c]
        n_e = hi - lo
        idx_flat = np.zeros(TT * 128, np.int32)
        dl_flat = np.zeros(TT * 128, f32)
        w_flat = np.zeros(TT * 128, f32)
        pos = np.arange(n_e) - blk_starts[blk]
        slot = toff[blk] * 128 + pos
        idx_flat[slot] = table_row[lo:hi].astype(np.int32)
        dl_flat[slot] = (d_loc & 127).astype(f32)
        w_flat[slot] = edge_weight[lo:hi].astype(f32)
        idx_dev = np.ascontiguousarray(idx_flat.reshape(TT, 128).T)
        dl_dev = np.ascontiguousarray(dl_flat.reshape(TT, 128).T)
        w_dev = np.ascontiguousarray(w_flat.reshape(TT, 128).T)

        xf = np.zeros((NPC, N_FEAT), f32)
        xf[:NPC_RAW] = features[NPC_RAW * c:NPC_RAW * (c + 1)]
        xTc = np.ascontiguousarray(xf.T.reshape(FCH, 128, NPC)).astype(f16)

        in_maps.append({
            "xT": xTc, "idx": idx_dev, "dlc": dl_dev, "wts": w_dev,
            "A1": A1c, "B1": B1c, "B2": B2c, "A2": A2p,
            "iota": iota, "ident": ident,
        })
    return in_maps, tbs


last_exec_time_ns = None
last_results = None


def kernel(features, src, dst, edge_weight, W_gc1, W_gc2, W_th, W_th1,
           trace=False, tmpdir=None):
    global last_exec_time_ns, last_results
    from concourse.bass_utils import run_bass_kernel_spmd

    features = np.asarray(features, np.float32)
    src = np.asarray(src, np.int32)
    dst = np.asarray(dst, np.int32)
    edge_weight = np.asarray(edge_weight, np.float32)
    W_gc1 = np.asarray(W_gc1, np.float32)
    W_gc2 = np.asarray(W_gc2, np.float32)
    W_th = np.asarray(W_th, np.float32)
    W_th1 = np.asarray(W_th1, np.float32)

    in_maps, tbs = _prep_host(features, src, dst, edge_weight,
                              W_gc1, W_gc2, W_th, W_th1)

    key = tuple(int(t) for t in tbs)
    if key not in _compiled_cache:
        _compiled_cache[key] = _build(NC, tbs)
    nc = _compiled_cache[key]

    res = run_bass_kernel_spmd(nc, in_maps, list(range(NC)),
                               trace=trace, tmpdir=tmpdir)
    last_exec_time_ns = res.exec_time_ns
    last_results = res

    h1 = np.empty((N_NODES, HID), np.float32)
    h2 = np.empty((N_NODES, HID), np.float32)
    for c in range(NC):
        h1[NPC_RAW * c:NPC_RAW * (c + 1)] = res.results[c]["out1"][:NPC_RAW]
        h2[NPC_RAW * c:NPC_RAW * (c + 1)] = res.results[c]["out2"][:NPC_RAW]
    return (h1, h2)
